# revision 1
# baseline (speedup 1.0000x reference)
"""Causal self-attention TRN2 kernel (8 NeuronCores).

Problem: x[4,2048,1024] f32, w_qkv[3072,1024], w_proj[1024,1024]
  qkv = x @ w_qkv.T; per-head causal softmax(q k^T / sqrt(64)) v; out @ w_proj.T

Sharding: 8 cores = (head-group hg in {0,1}) x (batch b in {0..3}).
  Core computes its 8 heads for its batch; partial y (contracted over its
  512 channels of w_proj input dim) is summed pairwise on host.

Per-core dataflow (all matmul inputs float32r = full-rate TF32-like):
  Stage 1: q,k projections -> qkT [1024, 2048] (f on partitions).
  Stage 2 (fused, per t-chunk): v projection -> V [2048, 8x65] (natural;
    col 65h+64 = ones giving the softmax denominator for free in PV),
    then attention for that i-block:
      S^T tiles [j=128, i=2x512] via PE (K=64, even/odd head row-tiled so
      the two MMs run concurrently), exp via ACT (scale=1/8) psum->sbuf,
      causal mask on straddling tiles via gpsimd.affine_select,
      PV via PE lhsT=[V|1] -> psum [65,512] (row 64 = denominator),
      normalize: DVE recip-approx + gpsimd partition_broadcast + DVE mul
      -> attnT [512, 2048]  (c_local on partitions)
    projection (attnT.T @ w_projT) interleaved lazily as PE filler.
"""

import numpy as np

import concourse.bacc as bacc
import concourse.mybir as mybir
import concourse.tile as tile
from concourse.bass_utils import run_bass_kernel_spmd

F32 = mybir.dt.float32
F32R = mybir.dt.float32r
EXP = mybir.ActivationFunctionType.Exp

B, T, C = 4, 2048, 1024
NH, HD = 16, 64
HPC = 8                      # heads per core
FH = HPC * HD                # 512: per-core q/k/v feature width
NCORES = 8
LAG = 2                      # scores->PV software-pipeline depth (j-tiles)

_CACHE = {}


def build_nc():
    nc = bacc.Bacc()
    xT_d = nc.dram_tensor("xT", [C, T], F32R, kind="ExternalInput")
    wqkvT_d = nc.dram_tensor("wqkvT", [C, 3 * FH], F32R, kind="ExternalInput")
    wprojT_d = nc.dram_tensor("wprojT", [FH, C], F32R, kind="ExternalInput")
    y_d = nc.dram_tensor("y", [T, C], F32, kind="ExternalOutput")

    NKT = C // 128           # 8 c-tiles (contraction for qkv)
    NTT = T // 128           # 16 t-tiles
    NTC = T // 512           # 4 t-chunks / i-blocks

    with tile.TileContext(nc) as tc:
        with (
            # ---------------- persistent pools (whole kernel) --------------
            tc.tile_pool(name="qkt", bufs=1) as qkt_pool,
            tc.tile_pool(name="vp", bufs=1) as v_pool,
            tc.tile_pool(name="wproj", bufs=1) as wproj_pool,
        ):
            qkT = [qkt_pool.tile([128, T], F32R, tag=f"qkt{i}", name=f"qkt{i}")
                   for i in range(8)]
            v_sb = [v_pool.tile([128, HPC * 65], F32R, tag=f"v{i}", name=f"v{i}")
                    for i in range(NTT)]
            wprojT = wproj_pool.tile([128, 4 * C], F32R, tag="wp", name="wp")

            wv_pool_outer = tc.tile_pool(name="wv", bufs=1)
            wv_pool = wv_pool_outer.__enter__()
            wv = wv_pool.tile([128, NKT * FH], F32R, tag="wv", name="wv")

            # prewarm the ACT exp table set during stage 1 (first ACTIVATE
            # otherwise pays the ~2.7us PSEUDO_LOAD_ACT_FUNC_SET at the
            # start of the attention phase)
            warm = wv_pool.tile([1, 8], F32, tag="warm", name="warm")
            nc.vector.memset(warm[:], 0.0)
            nc.scalar.activation(warm[0:1, :], warm[0:1, :], EXP, scale=1.0)

            psum_outer = [tc.tile_pool(name="psM", bufs=1, space="PSUM"),
                          tc.tile_pool(name="psS", bufs=1, space="PSUM"),
                          tc.tile_pool(name="psPV", bufs=1, space="PSUM")]
            psM = psum_outer[0].__enter__()
            psS = psum_outer[1].__enter__()
            psPV = psum_outer[2].__enter__()

            # ---------------- stage 1: q,k projections ---------------------
            with (
                tc.tile_pool(name="wqk", bufs=1) as wqk_pool,
                tc.tile_pool(name="xc1", bufs=1) as x1_pool,
            ):
                wqk = wqk_pool.tile([128, NKT * 2 * FH], F32R, tag="wqk",
                                    name="wqk")
                for tcb in range(NTC):
                    xc = x1_pool.tile([128, NKT * 512], F32R, tag="xc", bufs=2,
                                      name=f"xc{tcb}")
                    if tcb == 0:
                        # 2-way split so the first accumulation group can
                        # start after ~1.5MB instead of the full 5MB
                        nc.sync.dma_start(
                            out=wqk[:].rearrange("p (k f) -> p k f",
                                                 k=NKT)[:, 0:2, :],
                            in_=wqkvT_d[0:256, 0:2 * FH].rearrange(
                                "(k p) f -> p k f", p=128))
                        nc.sync.dma_start(
                            out=xc[:].rearrange("p (k t) -> p k t",
                                                k=NKT)[:, 0:2, :],
                            in_=xT_d[0:256, 0:512].rearrange(
                                "(k p) t -> p k t", p=128))
                        nc.sync.dma_start(
                            out=wqk[:].rearrange("p (k f) -> p k f",
                                                 k=NKT)[:, 2:NKT, :],
                            in_=wqkvT_d[256:C, 0:2 * FH].rearrange(
                                "(k p) f -> p k f", p=128))
                        nc.sync.dma_start(
                            out=xc[:].rearrange("p (k t) -> p k t",
                                                k=NKT)[:, 2:NKT, :],
                            in_=xT_d[256:C, 0:512].rearrange(
                                "(k p) t -> p k t", p=128))
                    else:
                        nc.sync.dma_start(
                            out=xc[:].rearrange("p (k t) -> p k t", k=NKT),
                            in_=xT_d[0:C, tcb * 512:(tcb + 1) * 512].rearrange(
                                "(k p) t -> p k t", p=128))
                    if tcb == 1:
                        # prefetch v weights + proj weights during stage 1
                        nc.sync.dma_start(
                            out=wv[:].rearrange("p (k f) -> p k f", k=NKT),
                            in_=wqkvT_d[0:C, 2 * FH:3 * FH].rearrange(
                                "(k p) f -> p k f", p=128))
                        nc.sync.dma_start(
                            out=wprojT[:].rearrange("p (g f) -> p g f", g=4),
                            in_=wprojT_d[0:FH, :].rearrange(
                                "(g p) f -> p g f", p=128))
                    for fi in range(8):      # 0-3 q rows, 4-7 k rows
                        ps = psM.tile([128, 512], F32, tag="mm512", bufs=2,
                                      name=f"psqk{tcb}_{fi}")
                        for k in range(NKT):
                            nc.tensor.matmul(
                                ps[:],
                                wqk[:, k * 1024 + fi * 128:
                                    k * 1024 + (fi + 1) * 128],
                                xc[:, k * 512:(k + 1) * 512],
                                start=(k == 0), stop=(k == NKT - 1))
                        nc.vector.tensor_copy(
                            out=qkT[fi][:, tcb * 512:(tcb + 1) * 512], in_=ps[:])

            # ------- stage 2: v projection + attention + output proj -------
            with (
                tc.tile_pool(name="xc2", bufs=1) as x2_pool,
                tc.tile_pool(name="attnt", bufs=1) as attnt_pool,
                tc.tile_pool(name="pt", bufs=1) as pt_pool,
                tc.tile_pool(name="stage", bufs=1) as stage_pool,
                tc.tile_pool(name="nrm", bufs=1) as nrm_pool,
            ):
                attnT = [attnt_pool.tile([128, T], F32R, tag=f"at{g}",
                                         name=f"at{g}") for g in range(4)]

                def emit_proj(ti, fc):
                    po = psM.tile([128, 512], F32, tag="mm512", bufs=2,
                                  name=f"po{ti}_{fc}")
                    for g in range(4):
                        nc.tensor.matmul(
                            po[:],
                            attnT[g][:, ti * 128:(ti + 1) * 128],
                            wprojT[:, g * C + fc * 512:
                                   g * C + (fc + 1) * 512],
                            start=(g == 0), stop=(g == 3))
                    ot = stage_pool.tile([128, 512], F32, tag="ot",
                                         bufs=2, name=f"ot{ti}_{fc}")
                    nc.vector.tensor_copy(out=ot[:], in_=po[:])
                    nc.sync.dma_start(
                        out=y_d[ti * 128:(ti + 1) * 128,
                                fc * 512:(fc + 1) * 512],
                        in_=ot[:])

                def emit_vgroup(ti):
                    xc = x2_pool.tile([128, NKT * 128], F32R, tag="x2",
                                      bufs=2, name=f"x2_{ti}")
                    nc.sync.dma_start(
                        out=xc[:].rearrange("p (k t) -> p k t", k=NKT),
                        in_=xT_d[0:C, ti * 128:(ti + 1) * 128].rearrange(
                            "(k p) t -> p k t", p=128))
                    ps = psM.tile([128, 512], F32, tag="mm512", bufs=2,
                                  name=f"psv{ti}")
                    for k in range(NKT):
                        nc.tensor.matmul(ps[:],
                                         xc[:, k * 128:(k + 1) * 128],
                                         wv[:, k * FH:(k + 1) * FH],
                                         start=(k == 0), stop=(k == NKT - 1))
                    vt = v_sb[ti]
                    vv = vt[:].rearrange("p (h x) -> p h x", h=HPC)
                    nc.vector.memset(vt[:].bitcast(F32), 1.0)
                    nc.vector.tensor_copy(
                        out=vv[:, :, 0:64],
                        in_=ps[:].rearrange("p (h x) -> p h x", h=HPC))

                pending = []
                for tcb in range(NTC):
                    if tcb == 0:
                        for tl in range(4):
                            emit_vgroup(tl)

                    # ---- attention for i-block bi = tcb ----
                    bi = tcb
                    njt = 4 * bi + 4
                    for hp in range(4):          # head pair (2hp, 2hp+1)
                        if pending:
                            emit_proj(*pending.pop(0))
                        qt = qkT[hp]
                        kt = qkT[4 + hp]
                        pts = []
                        pvs = [psPV.tile([65, 512], F32, tag="pv", bufs=2,
                                         name=f"pv{bi}_{hp}_{par}")
                               for par in range(2)]

                        def emit_scores(jj, bi=bi, hp=hp, qt=qt, kt=kt,
                                        pts=pts):
                            sps = psS.tile([128, 1024], F32, tag="sps", bufs=2,
                                           name=f"sps{bi}_{hp}_{jj}")
                            # even head on PE rows 0-63, odd head on rows
                            # 64-127 -> the two MMs run concurrently
                            for par in range(2):
                                off = par * 64
                                nc.tensor.matmul(
                                    sps[:, par * 512:(par + 1) * 512],
                                    kt[off:off + 64, jj * 128:(jj + 1) * 128],
                                    qt[off:off + 64, bi * 512:(bi + 1) * 512],
                                    start=True, stop=True)
                            pt = pt_pool.tile([128, 1024], F32R, tag="pt",
                                              bufs=LAG + 2,
                                              name=f"pt{bi}_{hp}_{jj}")
                            if jj < 4 * bi:
                                nc.scalar.activation(pt[:], sps[:], EXP,
                                                     scale=0.125)
                            else:
                                st = stage_pool.tile([128, 1024], F32R,
                                                     tag="st", bufs=2,
                                                     name=f"st{bi}_{hp}_{jj}")
                                r0 = jj - 4 * bi
                                lo = 128 * r0
                                sv = sps[:].rearrange("p (b i) -> p b i", b=2)
                                tv = st[:].rearrange("p (b i) -> p b i", b=2)
                                nc.scalar.activation(tv[:, :, lo:512],
                                                     sv[:, :, lo:512], EXP,
                                                     scale=0.125)
                                nc.gpsimd.affine_select(
                                    out=pt[:].rearrange("p (b i) -> p b i", b=2),
                                    in_=st[:].rearrange("p (b i) -> p b i", b=2),
                                    compare_op=mybir.AluOpType.is_ge,
                                    fill=0.0,
                                    base=-128 * r0,
                                    pattern=[[0, 2], [1, 512]],
                                    channel_multiplier=-1,
                                )
                            pts.append(pt)

                        def emit_pv(jj, bi=bi, hp=hp, pts=pts, pvs=pvs,
                                    njt=njt):
                            for par in range(2):
                                h = 2 * hp + par
                                nc.tensor.matmul(
                                    pvs[par][:],
                                    v_sb[jj][:, h * 65:h * 65 + 65],
                                    pts[jj][:, par * 512:(par + 1) * 512],
                                    start=(jj == 0), stop=(jj == njt - 1))

                        # software pipeline: PV lags scores by LAG j-tiles
                        for jj in range(njt):
                            emit_scores(jj)
                            if jj >= LAG:
                                emit_pv(jj - LAG)
                        for jj in range(max(0, njt - LAG), njt):
                            emit_pv(jj)
                        # v projection of the NEXT t-chunk as PE filler over
                        # the normalize chain
                        if tcb < NTC - 1:
                            emit_vgroup((tcb + 1) * 4 + hp)
                        if pending:
                            emit_proj(*pending.pop(0))
                        for par in range(2):
                            pv = pvs[par]
                            den = nrm_pool.tile([1, 512], F32, tag="den",
                                                bufs=1, name=f"den{bi}_{hp}_{par}")
                            nc.vector.tensor_copy(out=den[0:1, :],
                                                  in_=pv[64:65, :])
                            rec = nrm_pool.tile([1, 512], F32, tag="rec",
                                                bufs=1, name=f"rec{bi}_{hp}_{par}")
                            nc.vector.reciprocal_approx_fast(
                                out=rec[0:1, :], in_=den[0:1, :])
                            bc = nrm_pool.tile([64, 512], F32, tag="bc",
                                               bufs=2, name=f"bc{bi}_{hp}_{par}")
                            nc.gpsimd.partition_broadcast(bc[:, :], rec[0:1, :])
                            nc.vector.tensor_mul(
                                out=attnT[hp][par * 64:par * 64 + 64,
                                              bi * 512:(bi + 1) * 512],
                                in0=pv[0:64, :], in1=bc[:, :])
                    # queue this i-block's projection groups
                    for tl in range(4):
                        for fc in range(2):
                            pending.append((bi * 4 + tl, fc))
                for pf in pending:
                    emit_proj(*pf)
            for p in reversed(psum_outer):
                p.__exit__(None, None, None)
            wv_pool_outer.__exit__(None, None, None)
    nc.compile()
    return nc


def _get_nc():
    if "nc" not in _CACHE:
        _CACHE["nc"] = build_nc()
    return _CACHE["nc"]


def kernel(x, w_qkv, w_proj, _trace=False):
    x = np.asarray(x, dtype=np.float32)
    w_qkv = np.asarray(w_qkv, dtype=np.float32)
    w_proj = np.asarray(w_proj, dtype=np.float32)

    nc = _get_nc()
    in_maps = []
    for c in range(NCORES):
        hg, b = c // 4, c % 4
        xT = np.ascontiguousarray(x[b].T)                       # [1024, 2048]
        rows = []
        for sec in range(3):                                     # q, k, v
            rows.append(w_qkv[sec * C + hg * FH: sec * C + (hg + 1) * FH])
        wqkvT = np.ascontiguousarray(np.concatenate(rows, 0).T)  # [1024, 1536]
        wprojT = np.ascontiguousarray(w_proj[:, hg * FH:(hg + 1) * FH].T)
        in_maps.append({"xT": xT, "wqkvT": wqkvT, "wprojT": wprojT})

    res = run_bass_kernel_spmd(nc, in_maps, list(range(NCORES)), trace=_trace)
    if _trace:
        _CACHE["exec_time_ns"] = res.exec_time_ns

    y = np.empty((B, T, C), dtype=np.float32)
    for b in range(B):
        y[b] = res.results[b]["y"] + res.results[4 + b]["y"]
    return y



# revision 2
# speedup vs baseline: 1.0072x; 1.0072x over previous
"""Causal self-attention TRN2 kernel (8 NeuronCores).

Problem: x[4,2048,1024] f32, w_qkv[3072,1024], w_proj[1024,1024]
  qkv = x @ w_qkv.T; per-head causal softmax(q k^T / sqrt(64)) v; out @ w_proj.T

Sharding: 8 cores = (head-group hg in {0,1}) x (batch b in {0..3}).
  Core computes its 8 heads for its batch; partial y (contracted over its
  512 channels of w_proj input dim) is summed pairwise on host.

Per-core dataflow (all matmul inputs float32r = full-rate TF32-like):
  Stage 1: q,k projections -> qkT [1024, 2048] (f on partitions).
  Stage 2 (fused, per t-chunk): v projection -> V [2048, 8x65] (natural;
    col 65h+64 = ones giving the softmax denominator for free in PV),
    then attention for that i-block:
      S^T tiles [j=128, i=2x512] via PE (K=64, even/odd head row-tiled so
      the two MMs run concurrently), exp via ACT (scale=1/8) psum->sbuf,
      causal mask on straddling tiles via gpsimd.affine_select,
      PV via PE lhsT=[V|1] -> psum [65,512] (row 64 = denominator),
      normalize: DVE recip-approx + gpsimd partition_broadcast + DVE mul
      -> attnT [512, 2048]  (c_local on partitions)
    projection (attnT.T @ w_projT) interleaved lazily as PE filler.
"""

import numpy as np

import concourse.bacc as bacc
import concourse.mybir as mybir
import concourse.tile as tile
from concourse.bass_utils import run_bass_kernel_spmd

F32 = mybir.dt.float32
F32R = mybir.dt.float32r
EXP = mybir.ActivationFunctionType.Exp

B, T, C = 4, 2048, 1024
NH, HD = 16, 64
HPC = 8                      # heads per core
FH = HPC * HD                # 512: per-core q/k/v feature width
NCORES = 8
LAG = 2                      # scores->PV software-pipeline depth (j-tiles)

_CACHE = {}


def build_nc():
    nc = bacc.Bacc()
    xT_d = nc.dram_tensor("xT", [C, T], F32R, kind="ExternalInput")
    wqkvT_d = nc.dram_tensor("wqkvT", [C, 3 * FH], F32R, kind="ExternalInput")
    wprojT_d = nc.dram_tensor("wprojT", [FH, C], F32R, kind="ExternalInput")
    y_d = nc.dram_tensor("y", [T, C], F32, kind="ExternalOutput")

    NKT = C // 128           # 8 c-tiles (contraction for qkv)
    NTT = T // 128           # 16 t-tiles
    NTC = T // 512           # 4 t-chunks / i-blocks

    with tile.TileContext(nc) as tc:
        with (
            # ---------------- persistent pools (whole kernel) --------------
            tc.tile_pool(name="qkt", bufs=1) as qkt_pool,
            tc.tile_pool(name="vp", bufs=1) as v_pool,
            tc.tile_pool(name="wproj", bufs=1) as wproj_pool,
        ):
            qkT = [qkt_pool.tile([128, T], F32R, tag=f"qkt{i}", name=f"qkt{i}")
                   for i in range(8)]
            v_sb = [v_pool.tile([128, HPC * 65], F32R, tag=f"v{i}", name=f"v{i}")
                    for i in range(NTT)]
            wprojT = wproj_pool.tile([128, 4 * C], F32R, tag="wp", name="wp")

            wv_pool_outer = tc.tile_pool(name="wv", bufs=1)
            wv_pool = wv_pool_outer.__enter__()
            wv = wv_pool.tile([128, NKT * FH], F32R, tag="wv", name="wv")

            # prewarm the ACT exp table set during stage 1 (first ACTIVATE
            # otherwise pays the ~2.7us PSEUDO_LOAD_ACT_FUNC_SET at the
            # start of the attention phase)
            warm = wv_pool.tile([1, 8], F32, tag="warm", name="warm")
            nc.vector.memset(warm[:], 0.0)
            nc.scalar.activation(warm[0:1, :], warm[0:1, :], EXP, scale=1.0)

            psum_outer = [tc.tile_pool(name="psM", bufs=1, space="PSUM"),
                          tc.tile_pool(name="psS", bufs=1, space="PSUM"),
                          tc.tile_pool(name="psPV", bufs=1, space="PSUM")]
            psM = psum_outer[0].__enter__()
            psS = psum_outer[1].__enter__()
            psPV = psum_outer[2].__enter__()

            # ---------------- stage 1: q,k projections ---------------------
            with (
                tc.tile_pool(name="wqk", bufs=1) as wqk_pool,
                tc.tile_pool(name="xc1", bufs=1) as x1_pool,
            ):
                wqk = wqk_pool.tile([128, NKT * 2 * FH], F32R, tag="wqk",
                                    name="wqk")
                for tcb in range(NTC):
                    xc = x1_pool.tile([128, NKT * 512], F32R, tag="xc", bufs=2,
                                      name=f"xc{tcb}")
                    if tcb == 0:
                        # 2-way split so the first accumulation group can
                        # start after ~1.5MB instead of the full 5MB
                        nc.sync.dma_start(
                            out=wqk[:].rearrange("p (k f) -> p k f",
                                                 k=NKT)[:, 0:2, :],
                            in_=wqkvT_d[0:256, 0:2 * FH].rearrange(
                                "(k p) f -> p k f", p=128))
                        nc.sync.dma_start(
                            out=xc[:].rearrange("p (k t) -> p k t",
                                                k=NKT)[:, 0:2, :],
                            in_=xT_d[0:256, 0:512].rearrange(
                                "(k p) t -> p k t", p=128))
                        nc.sync.dma_start(
                            out=wqk[:].rearrange("p (k f) -> p k f",
                                                 k=NKT)[:, 2:NKT, :],
                            in_=wqkvT_d[256:C, 0:2 * FH].rearrange(
                                "(k p) f -> p k f", p=128))
                        nc.sync.dma_start(
                            out=xc[:].rearrange("p (k t) -> p k t",
                                                k=NKT)[:, 2:NKT, :],
                            in_=xT_d[256:C, 0:512].rearrange(
                                "(k p) t -> p k t", p=128))
                    else:
                        nc.sync.dma_start(
                            out=xc[:].rearrange("p (k t) -> p k t", k=NKT),
                            in_=xT_d[0:C, tcb * 512:(tcb + 1) * 512].rearrange(
                                "(k p) t -> p k t", p=128))
                    if tcb == 1:
                        # prefetch v weights + proj weights during stage 1
                        nc.sync.dma_start(
                            out=wv[:].rearrange("p (k f) -> p k f", k=NKT),
                            in_=wqkvT_d[0:C, 2 * FH:3 * FH].rearrange(
                                "(k p) f -> p k f", p=128))
                        nc.sync.dma_start(
                            out=wprojT[:].rearrange("p (g f) -> p g f", g=4),
                            in_=wprojT_d[0:FH, :].rearrange(
                                "(g p) f -> p g f", p=128))
                    for fi in range(8):      # 0-3 q rows, 4-7 k rows
                        ps = psM.tile([128, 512], F32, tag="mm512", bufs=2,
                                      name=f"psqk{tcb}_{fi}")
                        for k in range(NKT):
                            nc.tensor.matmul(
                                ps[:],
                                wqk[:, k * 1024 + fi * 128:
                                    k * 1024 + (fi + 1) * 128],
                                xc[:, k * 512:(k + 1) * 512],
                                start=(k == 0), stop=(k == NKT - 1))
                        nc.vector.tensor_copy(
                            out=qkT[fi][:, tcb * 512:(tcb + 1) * 512], in_=ps[:])

            # ------- stage 2: v projection + attention + output proj -------
            with (
                tc.tile_pool(name="xc2", bufs=1) as x2_pool,
                tc.tile_pool(name="attnt", bufs=1) as attnt_pool,
                tc.tile_pool(name="pt", bufs=1) as pt_pool,
                tc.tile_pool(name="stage", bufs=1) as stage_pool,
                tc.tile_pool(name="nrm", bufs=1) as nrm_pool,
            ):
                attnT = [attnt_pool.tile([128, T], F32R, tag=f"at{g}",
                                         name=f"at{g}") for g in range(4)]

                def emit_proj(ti, fc):
                    po = psM.tile([128, 512], F32, tag="mm512", bufs=2,
                                  name=f"po{ti}_{fc}")
                    for g in range(4):
                        nc.tensor.matmul(
                            po[:],
                            attnT[g][:, ti * 128:(ti + 1) * 128],
                            wprojT[:, g * C + fc * 512:
                                   g * C + (fc + 1) * 512],
                            start=(g == 0), stop=(g == 3))
                    ot = stage_pool.tile([128, 512], F32, tag="ot",
                                         bufs=2, name=f"ot{ti}_{fc}")
                    nc.vector.tensor_copy(out=ot[:], in_=po[:])
                    nc.sync.dma_start(
                        out=y_d[ti * 128:(ti + 1) * 128,
                                fc * 512:(fc + 1) * 512],
                        in_=ot[:])

                def emit_vgroup(ti):
                    xc = x2_pool.tile([128, NKT * 128], F32R, tag="x2",
                                      bufs=2, name=f"x2_{ti}")
                    nc.sync.dma_start(
                        out=xc[:].rearrange("p (k t) -> p k t", k=NKT),
                        in_=xT_d[0:C, ti * 128:(ti + 1) * 128].rearrange(
                            "(k p) t -> p k t", p=128))
                    ps = psM.tile([128, 512], F32, tag="mm512", bufs=2,
                                  name=f"psv{ti}")
                    for k in range(NKT):
                        nc.tensor.matmul(ps[:],
                                         xc[:, k * 128:(k + 1) * 128],
                                         wv[:, k * FH:(k + 1) * FH],
                                         start=(k == 0), stop=(k == NKT - 1))
                    vt = v_sb[ti]
                    vv = vt[:].rearrange("p (h x) -> p h x", h=HPC)
                    nc.vector.memset(vt[:].bitcast(F32), 1.0)
                    nc.vector.tensor_copy(
                        out=vv[:, :, 0:64],
                        in_=ps[:].rearrange("p (h x) -> p h x", h=HPC))

                pending = []
                for tcb in range(NTC):
                    if tcb == 0:
                        for tl in range(4):
                            emit_vgroup(tl)

                    # ---- attention for i-block bi = tcb ----
                    bi = tcb
                    njt = 4 * bi + 4
                    for hp in range(4):          # head pair (2hp, 2hp+1)
                        if pending:
                            emit_proj(*pending.pop(0))
                        qt = qkT[hp]
                        kt = qkT[4 + hp]
                        pts = []
                        pvs = [psPV.tile([65, 512], F32, tag="pv", bufs=2,
                                         name=f"pv{bi}_{hp}_{par}")
                               for par in range(2)]

                        def emit_scores(jj, bi=bi, hp=hp, qt=qt, kt=kt,
                                        pts=pts):
                            sps = psS.tile([128, 1024], F32, tag="sps", bufs=2,
                                           name=f"sps{bi}_{hp}_{jj}")
                            # even head on PE rows 0-63, odd head on rows
                            # 64-127 -> the two MMs run concurrently
                            for par in range(2):
                                off = par * 64
                                nc.tensor.matmul(
                                    sps[:, par * 512:(par + 1) * 512],
                                    kt[off:off + 64, jj * 128:(jj + 1) * 128],
                                    qt[off:off + 64, bi * 512:(bi + 1) * 512],
                                    start=True, stop=True)
                            pt = pt_pool.tile([128, 1024], F32R, tag="pt",
                                              bufs=LAG + 2,
                                              name=f"pt{bi}_{hp}_{jj}")
                            if jj < 4 * bi:
                                nc.scalar.activation(pt[:], sps[:], EXP,
                                                     scale=0.125)
                            else:
                                st = stage_pool.tile([128, 1024], F32R,
                                                     tag="st", bufs=2,
                                                     name=f"st{bi}_{hp}_{jj}")
                                r0 = jj - 4 * bi
                                lo = 128 * r0
                                sv = sps[:].rearrange("p (b i) -> p b i", b=2)
                                tv = st[:].rearrange("p (b i) -> p b i", b=2)
                                nc.scalar.activation(tv[:, :, lo:512],
                                                     sv[:, :, lo:512], EXP,
                                                     scale=0.125)
                                nc.gpsimd.affine_select(
                                    out=pt[:].rearrange("p (b i) -> p b i", b=2),
                                    in_=st[:].rearrange("p (b i) -> p b i", b=2),
                                    compare_op=mybir.AluOpType.is_ge,
                                    fill=0.0,
                                    base=-128 * r0,
                                    pattern=[[0, 2], [1, 512]],
                                    channel_multiplier=-1,
                                )
                            pts.append(pt)

                        def emit_pv(jj, bi=bi, hp=hp, pts=pts, pvs=pvs,
                                    njt=njt):
                            for par in range(2):
                                h = 2 * hp + par
                                nc.tensor.matmul(
                                    pvs[par][:],
                                    v_sb[jj][:, h * 65:h * 65 + 65],
                                    pts[jj][:, par * 512:(par + 1) * 512],
                                    start=(jj == 0), stop=(jj == njt - 1))

                        # software pipeline: PV lags scores by LAG j-tiles
                        for jj in range(njt):
                            emit_scores(jj)
                            if jj >= LAG:
                                emit_pv(jj - LAG)
                        for jj in range(max(0, njt - LAG), njt):
                            emit_pv(jj)
                        # v projection of the NEXT t-chunk as PE filler over
                        # the normalize chain
                        if tcb < NTC - 1:
                            emit_vgroup((tcb + 1) * 4 + hp)
                        if pending:
                            emit_proj(*pending.pop(0))
                        for par in range(2):
                            pv = pvs[par]
                            den = nrm_pool.tile([1, 512], F32, tag="den",
                                                bufs=1, name=f"den{bi}_{hp}_{par}")
                            nc.vector.tensor_copy(out=den[0:1, :],
                                                  in_=pv[64:65, :])
                            rec = nrm_pool.tile([1, 512], F32, tag="rec",
                                                bufs=1, name=f"rec{bi}_{hp}_{par}")
                            nc.vector.reciprocal_approx_fast(
                                out=rec[0:1, :], in_=den[0:1, :])
                            bc = nrm_pool.tile([64, 512], F32, tag="bc",
                                               bufs=2, name=f"bc{bi}_{hp}_{par}")
                            nc.gpsimd.partition_broadcast(bc[:, :], rec[0:1, :])
                            nc.vector.tensor_mul(
                                out=attnT[hp][par * 64:par * 64 + 64,
                                              bi * 512:(bi + 1) * 512],
                                in0=pv[0:64, :], in1=bc[:, :])
                    # queue this i-block's projection groups
                    for tl in range(4):
                        for fc in range(2):
                            pending.append((bi * 4 + tl, fc))
                for pf in pending:
                    emit_proj(*pf)
            for p in reversed(psum_outer):
                p.__exit__(None, None, None)
            wv_pool_outer.__exit__(None, None, None)
    nc.compile()
    return nc


def _get_nc():
    if "nc" not in _CACHE:
        _CACHE["nc"] = build_nc()
    return _CACHE["nc"]


def kernel(x, w_qkv, w_proj, _trace=False):
    x = np.asarray(x, dtype=np.float32)
    w_qkv = np.asarray(w_qkv, dtype=np.float32)
    w_proj = np.asarray(w_proj, dtype=np.float32)

    nc = _get_nc()
    in_maps = []
    for c in range(NCORES):
        hg, b = c // 4, c % 4
        xT = np.ascontiguousarray(x[b].T)                       # [1024, 2048]
        rows = []
        for sec in range(3):                                     # q, k, v
            rows.append(w_qkv[sec * C + hg * FH: sec * C + (hg + 1) * FH])
        wqkvT = np.ascontiguousarray(np.concatenate(rows, 0).T)  # [1024, 1536]
        wprojT = np.ascontiguousarray(w_proj[:, hg * FH:(hg + 1) * FH].T)
        in_maps.append({"xT": xT, "wqkvT": wqkvT, "wprojT": wprojT})

    res = run_bass_kernel_spmd(nc, in_maps, list(range(NCORES)), trace=_trace)
    if _trace:
        _CACHE["exec_time_ns"] = res.exec_time_ns
        _CACHE["res"] = res

    y = np.empty((B, T, C), dtype=np.float32)
    for b in range(B):
        y[b] = res.results[b]["y"] + res.results[4 + b]["y"]
    return y



# revision 11
# speedup vs baseline: 1.2300x; 1.2212x over previous
"""Causal self-attention TRN2 kernel (8 NeuronCores), v2.

Problem: x[4,2048,1024] f32, w_qkv[3072,1024], w_proj[1024,1024]
  qkv = x @ w_qkv.T; per-head causal softmax(q k^T / sqrt(64)) v; out @ w_proj.T

Sharding: 8 cores = (head-group hg in {0,1}) x (batch b in {0..3}).
  Core computes its 8 heads for its batch; partial y (contracted over its
  512 channels of w_proj input dim) is summed pairwise on host.

v2 design (vs v1): single fused pass, all matmul operands bf16
(f32 psum accumulation), so the exp ACT stream (1 elem/lane/cycle
@1.2GHz - the co-bottleneck) hides under a dense PE stream:

  per t-chunk tcb (= attention i-block bi):
    qk-projection for the chunk (per head-pair, just-in-time),
    v-projection (reuses the same x chunk tile),
    attention j-loop (descending j):
      scores pair MM (2 heads row-tiled, concurrent)
      [diagonal tiles: causal mask folded into the scores PSUM group as
       a -240 bias MM (exp -> 0) + N-trimmed to the valid columns]
      exp via ACT psum->sbuf bf16
      PV: per-head M=64 MMs col-tiled to array halves (concurrent) +
          denominator row MMs (M=1) at tile_position (0,0)/(0,32),
          all four accumulating into 1.5 psum banks
    normalize: 2x reciprocal_approx_fast on the den rows, one gpsimd
      partition_broadcast [128,1024], 2 DVE muls -> attnT (bf16)
    out-projection of block bi queued as PE filler into block bi+1.

  Next-chunk qk / v-proj / prev-block proj matmuls are pumped from a
  pending queue between attention slots to keep PE busy (HAM warm).
"""

import numpy as np
import ml_dtypes

import concourse.bacc as bacc
import concourse.mybir as mybir
import concourse.tile as tile
from concourse.bass_utils import run_bass_kernel_spmd

F32 = mybir.dt.float32
BF16 = mybir.dt.bfloat16
EXP = mybir.ActivationFunctionType.Exp

B, T, C = 4, 2048, 1024
NH, HD = 16, 64
HPC = 8                      # heads per core
FH = HPC * HD                # 512: per-core q/k/v feature width
NCORES = 8
NKT = C // 128               # 8 contraction tiles
NTC = T // 512               # 4 t-chunks / i-blocks
LAG = 3                      # scores->PV software-pipeline depth (j-tiles)

_CACHE = {}


def build_nc():
    nc = bacc.Bacc()
    xT_d = nc.dram_tensor("xT", [C, T], BF16, kind="ExternalInput")
    wqkvT_d = nc.dram_tensor("wqkvT", [C, 3 * FH], BF16, kind="ExternalInput")
    wprojT_d = nc.dram_tensor("wprojT", [FH, C], BF16, kind="ExternalInput")
    cst_d = nc.dram_tensor("cst", [128, 384], BF16, kind="ExternalInput")
    y_d = nc.dram_tensor("y", [T, C], F32, kind="ExternalOutput")
    import os
    DBG = bool(os.environ.get("BASSDBG"))
    if DBG:
        dq_d = nc.dram_tensor("dbg_qk", [8 * 128, T], BF16, kind="ExternalOutput")
        dv_d = nc.dram_tensor("dbg_v", [16 * 128, HPC * 65], BF16, kind="ExternalOutput")
        dd_d = nc.dram_tensor("dbg_den", [16, 1024], F32, kind="ExternalOutput")
        da_d = nc.dram_tensor("dbg_at", [4 * 128, T], BF16, kind="ExternalOutput")

    with tile.TileContext(nc) as tc:
        with (
            tc.tile_pool(name="qkt", bufs=1) as qkt_pool,
            tc.tile_pool(name="vp", bufs=1) as v_pool,
            tc.tile_pool(name="at", bufs=1) as at_pool,
            tc.tile_pool(name="wq", bufs=1) as wq_pool,
            tc.tile_pool(name="wvp", bufs=1) as wv_pool,
            tc.tile_pool(name="wpj", bufs=1) as wp_pool,
            tc.tile_pool(name="cstp", bufs=1) as cst_pool,
            tc.tile_pool(name="xcp", bufs=1) as x_pool,
            tc.tile_pool(name="ptp", bufs=1) as pt_pool,
            tc.tile_pool(name="nrm", bufs=1) as nrm_pool,
            tc.tile_pool(name="otp", bufs=1) as ot_pool,
            tc.tile_pool(name="psS", bufs=1, space="PSUM") as psS,
            tc.tile_pool(name="psPV", bufs=1, space="PSUM") as psPV,
            tc.tile_pool(name="psD", bufs=1, space="PSUM") as psD,
            tc.tile_pool(name="psM", bufs=1, space="PSUM") as psM,
        ):
            qkT = [qkt_pool.tile([128, T], BF16, tag=f"qkt{i}", name=f"qkt{i}")
                   for i in range(8)]
            v_sb = [v_pool.tile([128, HPC * 65], BF16, tag=f"v{i}",
                            name=f"v{i}") for i in range(4 * NTC)]
            attnT = [at_pool.tile([128, T], BF16, tag=f"at{g}", name=f"at{g}")
                     for g in range(4)]
            wqk = wq_pool.tile([128, NKT * 1024], BF16, tag="wqk", name="wqk")
            wv = wv_pool.tile([128, NKT * FH], BF16, tag="wv", name="wv")
            wpj = wp_pool.tile([128, 4 * C], BF16, tag="wpj", name="wpj")
            cst = cst_pool.tile([128, 384], BF16, tag="cst", name="cst")
            ident = cst[:, 0:128]

            # prewarm the ACT exp table (first ACTIVATE otherwise pays the
            # ~2.7us PSEUDO_LOAD_ACT_FUNC_SET inside the attention loop)
            warm = nrm_pool.tile([1, 8], F32, tag="warm", name="warm")
            nc.vector.memset(warm[:], 0.0)
            nc.scalar.activation(warm[0:1, :], warm[0:1, :], EXP, scale=1.0)

            # ---- initial DMAs: sync ring = critical path, scalar = bulk ----
            wqk_v = wqk[:].rearrange("p (k f) -> p k f", k=NKT)
            xcs = {0: x_pool.tile([128, NKT * 512], BF16, tag="xc", bufs=2,
                                  name="xc0")}
            xc0_v = xcs[0][:].rearrange("p (k t) -> p k t", k=NKT)
            nc.sync.dma_start(
                out=wqk_v[:, 0:2, :],
                in_=wqkvT_d[0:256, 0:1024].rearrange("(k p) f -> p k f", p=128))
            nc.sync.dma_start(
                out=xc0_v[:, 0:2, :],
                in_=xT_d[0:256, 0:512].rearrange("(k p) t -> p k t", p=128))
            nc.sync.dma_start(
                out=wv[:].rearrange("p (k f) -> p k f", k=NKT),
                in_=wqkvT_d[0:C, 2 * FH:3 * FH].rearrange(
                    "(k p) f -> p k f", p=128))
            nc.sync.dma_start(out=cst[:], in_=cst_d[:, :])
            nc.scalar.dma_start(
                out=wqk_v[:, 2:NKT, :],
                in_=wqkvT_d[256:C, 0:1024].rearrange("(k p) f -> p k f", p=128))
            nc.scalar.dma_start(
                out=xc0_v[:, 2:NKT, :],
                in_=xT_d[256:C, 0:512].rearrange("(k p) t -> p k t", p=128))
            nc.scalar.dma_start(
                out=wpj[:].rearrange("p (g f) -> p g f", g=4),
                in_=wprojT_d[0:FH, :].rearrange("(g p) f -> p g f", p=128))

            # ---------------- emitters ----------------
            def emit_qk_fi(tcb, fi):
                xc = xcs[tcb]
                ps = psM.tile([128, 512], F32, tag="mmA", bufs=2,
                              name=f"psqk{tcb}_{fi}")
                for k in range(NKT):
                    nc.tensor.matmul(
                        ps[:],
                        wqk[:, k * 1024 + fi * 128:k * 1024 + (fi + 1) * 128],
                        xc[:, k * 512:(k + 1) * 512],
                        start=(k == 0), stop=(k == NKT - 1))
                nc.vector.tensor_copy(
                    out=qkT[fi][:, tcb * 512:(tcb + 1) * 512], in_=ps[:])

            def emit_v(tcb, ti):
                xc = xcs[tcb]
                dt_ = ti % 4
                ps = psM.tile([128, 512], F32, tag="mmA", bufs=2,
                              name=f"psv{ti}")
                for k in range(NKT):
                    nc.tensor.matmul(
                        ps[:],
                        xc[:, k * 512 + dt_ * 128:k * 512 + (dt_ + 1) * 128],
                        wv[:, k * FH:(k + 1) * FH],
                        start=(k == 0), stop=(k == NKT - 1))
                vt = v_sb[ti]
                nc.vector.memset(vt[:], 1.0)
                nc.vector.tensor_copy(
                    out=vt[:].rearrange("p (h x) -> p h x", h=HPC)[:, :, 0:64],
                    in_=ps[:].rearrange("p (h x) -> p h x", h=HPC))

            def emit_proj(ti, fc):
                ps = psM.tile([128, 512], F32, tag="mmA", bufs=2,
                              name=f"po{ti}_{fc}")
                for g in range(4):
                    nc.tensor.matmul(
                        ps[:],
                        attnT[g][:, ti * 128:(ti + 1) * 128],
                        wpj[:, g * C + fc * 512:g * C + (fc + 1) * 512],
                        start=(g == 0), stop=(g == 3))
                ot = ot_pool.tile([128, 512], F32, tag="ot", bufs=2,
                                  name=f"ot{ti}_{fc}")
                nc.vector.tensor_copy(out=ot[:], in_=ps[:])
                nc.sync.dma_start(
                    out=y_d[ti * 128:(ti + 1) * 128,
                            fc * 512:(fc + 1) * 512],
                    in_=ot[:])

            def emit_xc(tcb):
                xc = x_pool.tile([128, NKT * 512], BF16, tag="xc", bufs=2,
                                 name=f"xc{tcb}")
                nc.sync.dma_start(
                    out=xc[:].rearrange("p (k t) -> p k t", k=NKT),
                    in_=xT_d[0:C, tcb * 512:(tcb + 1) * 512].rearrange(
                        "(k p) t -> p k t", p=128))
                xcs[tcb] = xc

            # pending PE-filler queue: (tag, closure)
            pending = []

            def pump(n=1):
                for _ in range(n):
                    if pending:
                        pending.pop(0)[1]()

            def flush(tag=None):
                keep = []
                for tg, fn in pending:
                    if tag is None or tg == tag:
                        fn()
                    else:
                        keep.append((tg, fn))
                pending[:] = keep

            def attention_hp(bi, hp):
                njt = 4 * bi + 4
                qt, kt = qkT[hp], qkT[4 + hp]
                pv = psPV.tile([65, 1024], F32, tag="pv", bufs=1,
                               name=f"pv{bi}_{hp}")
                pts = {}

                def emit_scores(jj):
                    sps = psS.tile([128, 1024], F32, tag="sps", bufs=2,
                                   name=f"sps{bi}_{hp}_{jj}")
                    r0 = jj - 4 * bi
                    lo = 128 * r0 if r0 >= 0 else 0
                    if r0 >= 0:
                        # causal bias: copy the -240 strictly-lower-tri
                        # pattern through PE into both par halves, then
                        # accumulate the diagonal-square scores on top;
                        # the region right of the square starts fresh.
                        for par in range(2):
                            nc.tensor.matmul(
                                sps[:, par * 512 + lo:par * 512 + lo + 128],
                                ident,
                                cst[:, 128 + 128 * par:256 + 128 * par],
                                start=True, stop=False)
                        for par in range(2):
                            off = par * 64
                            nc.tensor.matmul(
                                sps[:, par * 512 + lo:par * 512 + lo + 128],
                                kt[off:off + 64, jj * 128:(jj + 1) * 128],
                                qt[off:off + 64,
                                   bi * 512 + lo:bi * 512 + lo + 128],
                                start=False, stop=True)
                        if lo + 128 < 512:
                            for par in range(2):
                                off = par * 64
                                nc.tensor.matmul(
                                    sps[:, par * 512 + lo + 128:
                                        (par + 1) * 512],
                                    kt[off:off + 64, jj * 128:(jj + 1) * 128],
                                    qt[off:off + 64,
                                       bi * 512 + lo + 128:(bi + 1) * 512],
                                    start=True, stop=True)
                    else:
                        for par in range(2):
                            off = par * 64
                            nc.tensor.matmul(
                                sps[:, par * 512:(par + 1) * 512],
                                kt[off:off + 64, jj * 128:(jj + 1) * 128],
                                qt[off:off + 64, bi * 512:(bi + 1) * 512],
                                start=True, stop=True)
                    pt = pt_pool.tile([128, 1024], BF16, tag="pt",
                                      bufs=LAG + 2, name=f"pt{bi}_{hp}_{jj}")
                    if r0 >= 0 and lo > 0:
                        spsv = sps[:].rearrange("p (b i) -> p b i", b=2)
                        ptv = pt[:].rearrange("p (b i) -> p b i", b=2)
                        nc.scalar.activation(ptv[:, :, lo:512],
                                             spsv[:, :, lo:512], EXP,
                                             scale=0.125)
                    else:
                        nc.scalar.activation(pt[:], sps[:], EXP, scale=0.125)
                    pts[jj] = pt

                def emit_pv(jj, first, last):
                    r0 = jj - 4 * bi
                    lo = 128 * r0 if r0 >= 0 else 0
                    pt = pts.pop(jj)
                    for par in range(2):
                        h = 2 * hp + par
                        nc.tensor.matmul(
                            pv[:, par * 512 + lo:(par + 1) * 512],
                            v_sb[jj][:, h * 65:h * 65 + 65],
                            pt[:, par * 512 + lo:(par + 1) * 512],
                            start=first, stop=last)

                # ascending j: the first (start=True) MMs of the pv/den
                # accumulation chains cover the full column range; the
                # N-trimmed diagonal tiles come last
                for t in range(njt + LAG):
                    if t < njt:
                        emit_scores(t)
                    if t >= LAG:
                        jj = t - LAG
                        emit_pv(jj, first=(jj == 0), last=(jj == njt - 1))
                    pump(1)

                # normalize: pv rows scaled by 1/den along queries
                dsb = nrm_pool.tile([1, 1024], F32, tag="dsb", bufs=2,
                                    name=f"dsb{bi}_{hp}")
                nc.vector.tensor_copy(out=dsb[0:1, :], in_=pv[64:65, :])
                if DBG:
                    nc.sync.dma_start(out=dd_d[4 * bi + hp:4 * bi + hp + 1, :],
                                      in_=dsb[0:1, :])
                rec = nrm_pool.tile([1, 1024], F32, tag="rec", bufs=2,
                                    name=f"rec{bi}_{hp}")
                nc.vector.reciprocal_approx_fast(out=rec[0:1, :],
                                                 in_=dsb[0:1, :])
                bc = nrm_pool.tile([128, 1024], F32, tag="bc", bufs=2,
                                   name=f"bc{bi}_{hp}")
                nc.gpsimd.partition_broadcast(bc[:, :], rec[0:1, :])
                nc.vector.tensor_mul(
                    out=attnT[hp][0:64, bi * 512:(bi + 1) * 512],
                    in0=pv[0:64, 0:512], in1=bc[0:64, 0:512])
                nc.vector.tensor_mul(
                    out=attnT[hp][64:128, bi * 512:(bi + 1) * 512],
                    in0=pv[0:64, 512:1024], in1=bc[0:64, 512:1024])

            # ---------------- main fused loop ----------------
            projq = []
            for tcb in range(NTC):
                bi = tcb
                emit_qk_fi(tcb, 0)
                emit_qk_fi(tcb, 4)
                for hp in range(4):
                    if hp == 0:
                        # ascending: emit_v(ti) must precede emit_pv(jj=ti),
                        # which lands at slot (ti - 4*tcb) + LAG
                        for ti in range(4 * tcb, 4 * tcb + 4):
                            pending.append(
                                ("v", lambda tcb=tcb, ti=ti: emit_v(tcb, ti)))
                    if hp < 3:
                        pending.append(
                            (f"qk{hp + 1}",
                             lambda tcb=tcb, fi=hp + 1: emit_qk_fi(tcb, fi)))
                        pending.append(
                            (f"qk{hp + 1}",
                             lambda tcb=tcb, fi=5 + hp: emit_qk_fi(tcb, fi)))
                    if hp == 1 and tcb < NTC - 1:
                        pending.append(
                            ("xc", lambda tcb=tcb: emit_xc(tcb + 1)))
                    for _ in range(2):
                        if projq:
                            pending.append(projq.pop(0))
                    flush(f"qk{hp}")   # normally a no-op
                    attention_hp(bi, hp)
                for ti in range(4 * bi, 4 * bi + 4):
                    for fc in range(2):
                        projq.append(
                            ("proj", lambda ti=ti, fc=fc: emit_proj(ti, fc)))
            flush()
            for _, fn in projq:
                fn()
            if DBG:
                for i in range(8):
                    nc.sync.dma_start(out=dq_d[i * 128:(i + 1) * 128, :], in_=qkT[i][:])
                for i in range(16):
                    nc.sync.dma_start(out=dv_d[i * 128:(i + 1) * 128, :],
                                      in_=v_sb[i][:])
                for g in range(4):
                    nc.sync.dma_start(out=da_d[g * 128:(g + 1) * 128, :], in_=attnT[g][:])
    nc.compile()
    return nc


def _get_nc():
    if "nc" not in _CACHE:
        _CACHE["nc"] = build_nc()
    return _CACHE["nc"]


def kernel(x, w_qkv, w_proj, _trace=False):
    x = np.asarray(x, dtype=np.float32)
    w_qkv = np.asarray(w_qkv, dtype=np.float32)
    w_proj = np.asarray(w_proj, dtype=np.float32)
    BF = ml_dtypes.bfloat16

    nc = _get_nc()

    r = np.arange(128)
    patt = np.where(r[None, :] < r[:, None], -240.0, 0.0).astype(np.float32)
    cstnp = np.concatenate([np.eye(128, dtype=np.float32), patt, patt],
                           axis=1).astype(BF)

    in_maps = []
    for c in range(NCORES):
        hg, b = c // 4, c % 4
        xT = np.ascontiguousarray(x[b].T).astype(BF)          # [1024, 2048]
        rows = []
        for sec in range(3):                                   # q, k, v
            rows.append(w_qkv[sec * C + hg * FH: sec * C + (hg + 1) * FH])
        wqkvT = np.ascontiguousarray(np.concatenate(rows, 0).T).astype(BF)
        wprojT = np.ascontiguousarray(
            w_proj[:, hg * FH:(hg + 1) * FH].T).astype(BF)
        in_maps.append({"xT": xT, "wqkvT": wqkvT, "wprojT": wprojT,
                        "cst": cstnp})

    res = run_bass_kernel_spmd(nc, in_maps, list(range(NCORES)), trace=_trace)
    if _trace:
        _CACHE["exec_time_ns"] = res.exec_time_ns
        _CACHE["res"] = res

    y = np.empty((B, T, C), dtype=np.float32)
    for b in range(B):
        y[b] = res.results[b]["y"] + res.results[4 + b]["y"]
    return y
